# revision 6
# baseline (speedup 1.0000x reference)
"""FFM pairwise-interaction kernel for Trainium2 (8 NeuronCores, batch-sharded).

out[b, p*64+e] = x[b, i, e] * x[b, j, e] * fe[i, j, e] * fe[j, i, e]
for the p-th pair (i, j), i < j, in row-major triu order.

Strategy (per core, batch shard of 512 rows):
  - batch lives on SBUF partitions (4 tiles of 128 rows).
  - inter[p, e] = fe[i,j,e] * fe[j,i,e] is computed on-device in partition 0
    and replicated to all 128 partitions with doubling SBUF->SBUF DMAs,
    streamed in chunks of whole pair-blocks (block i = pairs (i, i+1..39)).
  - pass 1 (VectorE): tmp = x_i (free-dim step-0 broadcast) * x_suffix
  - pass 2 (VectorE): out = tmp * inter_rep   (in place)
  - DMA out per (tile, chunk).
"""

import numpy as np

import concourse.bass as bass
import concourse.mybir as mybir
import concourse.tile as tile
from concourse import bacc, bass_utils

F32 = mybir.dt.float32

N_CORES = 8
B_FULL = 4096
F = 40
E = 64
B = B_FULL // N_CORES          # 512 rows per core
P = 128                        # SBUF partitions
N_TILES = B // P               # 4
PAIRS = F * (F - 1) // 2       # 780
OUT_COLS = PAIRS * E           # 49920

# column offset of each pair-block (block i covers pairs (i, j) j=i+1..F-1)
BLOCK_OFF = []
_off = 0
for _i in range(F - 1):
    BLOCK_OFF.append(_off)
    _off += (F - 1 - _i) * E
assert _off == OUT_COLS

CHUNK_CAP = 4160  # max columns per streamed chunk (65 pairs)


def _chunks():
    """Greedy grouping of whole blocks into chunks of <= CHUNK_CAP columns."""
    chunks = []
    cur_blocks, cur_cols = [], 0
    for i in range(F - 1):
        c = (F - 1 - i) * E
        if cur_blocks and cur_cols + c > CHUNK_CAP:
            chunks.append((BLOCK_OFF[cur_blocks[0]], cur_cols, cur_blocks))
            cur_blocks, cur_cols = [], 0
        cur_blocks.append(i)
        cur_cols += c
    chunks.append((BLOCK_OFF[cur_blocks[0]], cur_cols, cur_blocks))
    return chunks


CHUNKS = _chunks()


def build_nc() -> bass.Bass:
    nc = bacc.Bacc(
        "TRN2",
        target_bir_lowering=False,
        debug=False,
        enable_asserts=False,
        num_devices=N_CORES,
    )
    x = nc.dram_tensor("x", [B, F * E], F32, kind="ExternalInput")
    fe = nc.dram_tensor("feat_embedding", [F, F, E], F32, kind="ExternalInput")
    out = nc.dram_tensor("out", [B, OUT_COLS], F32, kind="ExternalOutput")

    with tile.TileContext(nc) as tc:
        with (
            tc.tile_pool(name="xp", bufs=1) as xp,
            tc.tile_pool(name="flatp", bufs=2) as flatp,
            tc.tile_pool(name="interp", bufs=2) as interp,
            tc.tile_pool(name="outp", bufs=3) as outp,
        ):
            # resident x tiles, one slot each
            x_sb = []
            for t in range(N_TILES):
                xt = xp.tile([P, F * E], F32, tag=f"x{t}")
                nc.sync.dma_start(out=xt[:], in_=x[t * P : (t + 1) * P, :])
                x_sb.append(xt)

            for coff, cols, blocks in CHUNKS:
                # ---- build replicated inter chunk [P, cols] ----
                rep = interp.tile([P, cols], F32, tag="rep")
                for b in blocks:
                    nq = F - 1 - b
                    seg = BLOCK_OFF[b] - coff
                    triu = flatp.tile([1, nq * E], F32, tag="triu")
                    tril = flatp.tile([1, nq * E], F32, tag="tril")
                    # fe[b, b+1:, :]  (contiguous)
                    nc.scalar.dma_start(
                        out=triu[0:1, :].rearrange("p (q e) -> p q e", e=E),
                        in_=fe[b : b + 1, b + 1 :, :],
                    )
                    # fe[b+1:, b, :]  (strided)
                    nc.scalar.dma_start(
                        out=tril[0:1, :].rearrange("p (q e) -> p q e", e=E),
                        in_=fe[b + 1 :, b, :].unsqueeze(0),
                    )
                    nc.vector.tensor_mul(
                        out=rep[0:1, seg : seg + nq * E],
                        in0=triu[0:1, :],
                        in1=tril[0:1, :],
                    )
                # broadcast partition 0 -> all partitions (doubling)
                k = 1
                while k < P:
                    kk = min(k, P - k)
                    nc.scalar.dma_start(out=rep[k : k + kk, :], in_=rep[0:kk, :])
                    k += kk

                # ---- per batch tile: pass1 + pass2 + store ----
                for t in range(N_TILES):
                    ob = outp.tile([P, cols], F32, tag="ob")
                    for b in blocks:
                        nq = F - 1 - b
                        seg = BLOCK_OFF[b] - coff
                        xi = (
                            x_sb[t][:, b * E : (b + 1) * E]
                            .unsqueeze(1)
                            .broadcast_to([P, nq, E])
                        )
                        xj = x_sb[t][:, (b + 1) * E : F * E].rearrange(
                            "p (q e) -> p q e", e=E
                        )
                        o = ob[:, seg : seg + nq * E].rearrange(
                            "p (q e) -> p q e", e=E
                        )
                        nc.vector.tensor_mul(out=o, in0=xi, in1=xj)
                    # pass 2, in place
                    nc.vector.tensor_mul(out=ob[:], in0=ob[:], in1=rep[:])
                    nc.sync.dma_start(
                        out=out[t * P : (t + 1) * P, coff : coff + cols],
                        in_=ob[:],
                    )
    nc.finalize()
    return nc


_NC = None


def _get_nc():
    global _NC
    if _NC is None:
        _NC = build_nc()
    return _NC


def kernel(x: np.ndarray, feat_embedding: np.ndarray, trace: bool = False):
    assert x.shape == (B_FULL, F, E) and feat_embedding.shape == (F, F, E)
    x = np.ascontiguousarray(x, dtype=np.float32).reshape(B_FULL, F * E)
    fe = np.ascontiguousarray(feat_embedding, dtype=np.float32)
    nc = _get_nc()
    in_maps = [
        {"x": x[c * B : (c + 1) * B], "feat_embedding": fe} for c in range(N_CORES)
    ]
    res = bass_utils.run_bass_kernel_spmd(
        nc, in_maps, core_ids=list(range(N_CORES)), trace=trace
    )
    kernel.last_result = res
    return np.concatenate([r["out"] for r in res.results], axis=0)
